# revision 23
# baseline (speedup 1.0000x reference)
"""Longformer sliding-window attention on 8 trn2 NeuronCores.

B=2, H=12, L=4096, D=64, one-sided window w=256 (full window 513).
Shard: 24 (b,h) pairs -> 3 heads per core.

Per-core algorithm (per head; 32 key-blocks of 128 keys):
  S^T[kb] = (128 keys x 640 queries) scores via bf16 matmuls
            (lhsT = K^T block (64,128), rhs = Q^T span (64,<=640)).
            Two key-blocks share one 3-bank PSUM tile (sub-tile stride
            768 f32) so one exp covers both; matmuls split at PSUM bank
            boundaries (sub-tile col 512 / 256).
  P^T     = exp(S/8) on ScalarE (PSUM->SBUF, bf16); band edges masked
            multiplicatively on VectorE (two 128x128 triangle masks).
  PV      : V2-stationary matmuls (lhsT = [V_kb | ones] (128 x 65),
            moving = P^T span) accumulating ctx^T into (65, 512) PSUM
            tiles: rows 0:64 = unnormalized ctx^T, row 64 = softmax
            denominator. 8 ctx tiles per head, ~8 contributing
            key-blocks each. Evacuated to bf16 SBUF on VectorE/ScalarE
            (alternating), DMA'd out unnormalized.
  norm    : on host -- ctx = ctx_u / denom (float32), plus transpose.

All DRAM I/O is bf16: per head 0.5 (qt) + 0.5 (kt) + 0.52 (v2) in,
0.53 (out) out.
"""

import sys

sys.path.insert(0, "/opt/trn_rl_repo")

import numpy as np
import ml_dtypes

B, H, L, D = 2, 12, 4096, 64
W = 256            # one-sided window
NCORES = 8
HPC = (B * H) // NCORES   # heads per core = 3
BLK = 128                 # key/query block (partition dim)
NB = L // BLK             # 32 key blocks per head
SPAN = 2 * W + BLK        # 640 query columns per key block
V2W = D + 1               # [V | ones] width
CTXW = 512                # ctx tile width (one PSUM bank)
NT = L // CTXW            # 8 ctx tiles per head

_CACHE = {}
REPEAT = 1                # python-unrolled repeat of the head loop
REPEAT_HW = 1             # hardware For_i repeat wrapping the head loop
_ABLATE = set()           # dev-only: subset of {"masks", "exp", "pv", "qkmin"}


def _build_program():
    import contextlib

    import concourse.bacc as bacc
    import concourse.bass as bass
    import concourse.mybir as mybir
    import concourse.tile as tile

    f32 = mybir.dt.float32
    bf16 = mybir.dt.bfloat16

    nc = bacc.Bacc("TRN2", target_bir_lowering=False, debug=False)

    qt_d = nc.dram_tensor("qt", [HPC, D, L], bf16, kind="ExternalInput").ap()
    kt_d = nc.dram_tensor("kt", [HPC, D, L], bf16, kind="ExternalInput").ap()
    v2_d = nc.dram_tensor("v2", [HPC, BLK, NB, V2W], bf16, kind="ExternalInput").ap()
    em_d = nc.dram_tensor("masks", [BLK, 2, BLK], bf16, kind="ExternalInput").ap()
    out_d = nc.dram_tensor("out", [HPC, V2W, L], bf16, kind="ExternalOutput").ap()

    # per-kb geometry
    geo = []
    for kb in range(NB):
        K0 = BLK * kb
        qbase = K0 - W
        qlo = max(0, qbase)
        qhi = min(L, K0 + BLK + W)
        geo.append((K0, qbase, qlo, qhi))

    first_kb = {T: max(0, 4 * T - 2) for T in range(NT)}
    last_kb = {T: min(NB - 1, 4 * T + 5) for T in range(NT)}

    with tile.TileContext(nc) as tc:
        with (
            tc.tile_pool(name="const", bufs=1) as constp,
            tc.tile_pool(name="qk", bufs=2) as qkp,
            tc.tile_pool(name="pt", bufs=4) as ptp,
            tc.tile_pool(name="outb", bufs=2) as outp,
            tc.tile_pool(name="st", bufs=2, space="PSUM") as stp,
            tc.tile_pool(name="ctx", bufs=2, space="PSUM") as ctxp,
        ):
            em = constp.tile([BLK, 2, BLK], bf16)
            nc.sync.dma_start(out=em, in_=em_d)

            loop_cm = (
                tc.For_i(0, REPEAT_HW) if REPEAT_HW > 1 else contextlib.nullcontext()
            )
            with loop_cm:
                for h in [hh for _ in range(REPEAT) for hh in range(HPC)]:
                    qt_t = qkp.tile([D, L], bf16)
                    nc.sync.dma_start(out=qt_t, in_=qt_d[h])
                    kt_t = qkp.tile([D, L], bf16)
                    nc.sync.dma_start(out=kt_t, in_=kt_d[h])
                    v2_t = qkp.tile([BLK, NB, V2W], bf16)
                    nc.sync.dma_start(out=v2_t, in_=v2_d[h])

                    outbuf = outp.tile([V2W, NT, CTXW], bf16)
                    cts = {}
                    pts = {}

                    def pv_mm(T, kk, start, stop):
                        # ctx^T accumulation: rows 0:64 ctx, row 64 denom.
                        # The first MM (kk == 4T+2) covers the tile's full
                        # 512 cols, so every later MM accumulates into an
                        # already-written range (uniform has_written state).
                        qlo_, qhi_, qbase_ = geo[kk][2], geo[kk][3], geo[kk][1]
                        a = max(qlo_, CTXW * T)
                        b = min(qhi_, CTXW * (T + 1))
                        pp, ss = pts[kk]
                        nc.tensor.matmul(
                            cts[T][:, a - CTXW * T : b - CTXW * T],
                            v2_t[:, kk, :],
                            pp[:, ss, a - qbase_ : b - qbase_],
                            start=start,
                            stop=stop,
                        )
                        if stop:
                            ct = cts.pop(T)
                            if T % 2 == 0:
                                nc.vector.tensor_copy(outbuf[:, T, :], ct)
                            else:
                                nc.scalar.copy(outbuf[:, T, :], ct)

                    for kb in range(NB):
                        K0, qbase, qlo, qhi = geo[kb]
                        c_lo, c_hi = qlo - qbase, qhi - qbase
                        s = kb % 2

                        if s == 0:
                            # sub-tile stride 768 f32 = 1.5 banks so the pair
                            # tile is exactly 3 banks; matmul outputs must stay
                            # within one 2KiB PSUM bank, and bank boundaries
                            # fall at sub-tile col 512 (s=0) / 256 (s=1).
                            st = stp.tile([BLK, 2, 768], f32, name="st", tag="st")
                            pt = ptp.tile([BLK, 2, SPAN], bf16, name="pt", tag="pt")
                        split = 512 - s * 256
                        qkw = 8 if "qkmin" in _ABLATE else SPAN
                        if c_lo < split:
                            a, b = c_lo, min(split, c_hi, c_lo + qkw)
                            nc.tensor.matmul(
                                st[:, s, a:b],
                                kt_t[:, K0 : K0 + BLK],
                                qt_t[:, qbase + a : qbase + b],
                                start=True,
                                stop=True,
                            )
                        if c_hi > split:
                            nc.tensor.matmul(
                                st[:, s, split : min(c_hi, split + qkw)],
                                kt_t[:, K0 : K0 + BLK],
                                qt_t[:, qbase + split : qbase + min(c_hi, split + qkw)],
                                start=True,
                                stop=True,
                            )

                        if s == 1:
                            def crange(kk):
                                K0_, qbase_, qlo_, qhi_ = geo[kk]
                                return qlo_ - qbase_, qhi_ - qbase_

                            cl0, ch0 = crange(kb - 1)
                            cl1, ch1 = crange(kb)
                            if "exp" in _ABLATE:
                                pass
                            elif (cl0, ch0) == (0, SPAN) and (cl1, ch1) == (0, SPAN):
                                # both sub-tiles full-span: one exp for the pair
                                nc.scalar.activation(
                                    pt[:, :, :],
                                    st[:, :, 0:SPAN],
                                    mybir.ActivationFunctionType.Exp,
                                    scale=float(1.0 / np.sqrt(D)),
                                )
                            else:
                                for ss, (cl, ch) in ((0, (cl0, ch0)), (1, (cl1, ch1))):
                                    nc.scalar.activation(
                                        pt[:, ss, cl:ch],
                                        st[:, ss, cl:ch],
                                        mybir.ActivationFunctionType.Exp,
                                        scale=float(1.0 / np.sqrt(D)),
                                    )

                            for kk, (cl, ch) in (
                                (kb - 1, (cl0, ch0)),
                                (kb, (cl1, ch1)),
                            ):
                                sub = pt[:, kk % 2, :]
                                if "masks" in _ABLATE:
                                    pass
                                elif (cl, ch) == (0, SPAN):
                                    # both triangle masks in one strided op
                                    pte = bass.AP(
                                        tensor=sub.tensor,
                                        offset=sub.offset,
                                        ap=[sub.ap[0], [4 * BLK, 2], [1, BLK]],
                                    )
                                    nc.vector.tensor_mul(pte, pte, em)
                                elif cl == 0:
                                    nc.vector.tensor_mul(
                                        sub[:, 0:BLK], sub[:, 0:BLK], em[:, 0, :]
                                    )
                                elif ch == SPAN:
                                    nc.vector.tensor_mul(
                                        sub[:, 4 * BLK : SPAN],
                                        sub[:, 4 * BLK : SPAN],
                                        em[:, 1, :],
                                    )

                            # PV: V2-stationary, ctx^T accumulation per 512-q
                            # tile. Tile T's burst starts at kk = 4T+2 (its
                            # full-coverage key-block, start=True), then the
                            # buffered earlier key-blocks accumulate; later
                            # key-blocks accumulate as they arrive.
                            for kk in (kb - 1, kb):
                                pts[kk] = (pt, kk % 2)
                            for kk in (kb - 1, kb):
                                if (kk - 2) % 4 == 0 and 0 <= (kk - 2) // 4 < NT:
                                    T = (kk - 2) // 4
                                    cts[T] = ctxp.tile(
                                        [V2W, CTXW], f32, name="ct", tag="ct"
                                    )
                                    pv_mm(T, kk, start=True, stop=False)
                                    for kj in range(first_kb[T], kk):
                                        if "pv" in _ABLATE:
                                            continue
                                        pv_mm(T, kj, start=False, stop=False)
                                for T in range((kk - 5 + 3) // 4, (kk + 2) // 4 + 1):
                                    if 0 <= T < NT and 4 * T + 2 < kk <= last_kb[T]:
                                        if "pv" in _ABLATE and kk != last_kb[T]:
                                            continue
                                        pv_mm(
                                            T, kk, start=False, stop=kk == last_kb[T]
                                        )
                            for old in [
                                kj for kj in pts if kj < kb - 6
                            ]:
                                del pts[old]

                    pts.clear()
                    nc.sync.dma_start(out=out_d[h], in_=outbuf)

    nc.compile()
    return nc


def _get_nc():
    if "nc" not in _CACHE:
        _CACHE["nc"] = _build_program()
    return _CACHE["nc"]


def _host_prep(q, k, v):
    bf = ml_dtypes.bfloat16
    BH = B * H
    qf = (
        np.asarray(q, dtype=np.float32)
        .transpose(0, 1, 3, 2)
        .reshape(BH, D, L)
        .astype(bf)
    )
    kf = (
        np.asarray(k, dtype=np.float32)
        .transpose(0, 1, 3, 2)
        .reshape(BH, D, L)
        .astype(bf)
    )
    v2 = np.empty((BH, BLK, NB, V2W), dtype=bf)
    v2[..., :D] = (
        np.asarray(v, dtype=np.float32)
        .reshape(BH, NB, BLK, D)
        .transpose(0, 2, 1, 3)
        .astype(bf)
    )
    v2[..., D] = 1.0

    i = np.arange(BLK)
    em = np.zeros((BLK, 2, BLK), dtype=bf)
    em[:, 0, :] = (i[None, :] >= i[:, None]).astype(bf)  # left edge: col>=row
    em[:, 1, :] = (i[None, :] <= i[:, None]).astype(bf)  # right edge: col<=row

    in_maps = []
    for c in range(NCORES):
        sl = slice(c * HPC, (c + 1) * HPC)
        in_maps.append(
            {
                "qt": np.ascontiguousarray(qf[sl]),
                "kt": np.ascontiguousarray(kf[sl]),
                "v2": np.ascontiguousarray(v2[sl]),
                "masks": em,
            }
        )
    return in_maps


def kernel(q, k, v, padding_mask):
    from concourse.bass_utils import run_bass_kernel_spmd

    pm = np.asarray(padding_mask)
    assert pm.all(), "kernel specialized for all-ones padding mask"

    nc = _get_nc()
    in_maps = _host_prep(q, k, v)
    res = run_bass_kernel_spmd(nc, in_maps, core_ids=list(range(NCORES)))
    outs = [res.results[c]["out"] for c in range(NCORES)]  # (HPC, 65, L)
    full = np.concatenate(outs, axis=0).astype(np.float32)  # (24, 65, L)
    ctx = full[:, :D, :] / full[:, D : D + 1, :]
    ctx = ctx.transpose(0, 2, 1).reshape(B, H, L, D)
    return np.ascontiguousarray(ctx.astype(np.float32))


# revision 25
# speedup vs baseline: 1.2377x; 1.2377x over previous
"""Longformer sliding-window attention on 8 trn2 NeuronCores.

B=2, H=12, L=4096, D=64, one-sided window w=256 (full window 513).
Shard: 24 (b,h) pairs -> 3 heads per core.

Per-core algorithm (per head; 32 key-blocks of 128 keys = 32 query-blocks):
  S^T[kb] = (128 keys x 640 queries) scores via bf16 matmuls
            (lhsT = K^T block (64,128), rhs = Q^T span (64,<=640)),
            split at PSUM-bank col 512.
  P^T     = exp(S/8) on ScalarE (PSUM->SBUF, bf16); band edges masked
            multiplicatively on VectorE (two 128x128 triangle masks).
  PV      : probs-stationary matmuls. For query-block t, 5 accumulating
            matmuls (kb = t-2..t+2): lhsT = P^T[kb][:, off:off+128]
            (128 keys x 128 queries), rhs = [V_kb | ones] (128 x 65)
            -> PSUM (128 queries x 65): cols 0:64 = ctx, col 64 = denom.
            Groups of 4 query-blocks share one PSUM bank; each block's
            accumulation group runs back-to-back (bank-safe).
  norm    : denom is a per-partition column -> reciprocal_approx_fast
            (128 x <=4) + one broadcast tensor_mul -> bf16 SBUF -> DMA.

All DRAM I/O is bf16: per head 0.5 (qt) + 0.5 (kt) + 0.52 (v2) in,
0.5 (out) out. Output assembled host-side (transpose + f32 cast only).
"""

import sys

sys.path.insert(0, "/opt/trn_rl_repo")

import numpy as np
import ml_dtypes

B, H, L, D = 2, 12, 4096, 64
W = 256            # one-sided window
NCORES = 8
HPC = (B * H) // NCORES   # heads per core = 3
BLK = 128                 # key/query block (partition dim)
NB = L // BLK             # 32 blocks per head
SPAN = 2 * W + BLK        # 640 query columns per key block
V2W = D + 1               # [V | ones] width
GRP = 4                   # query-blocks per PSUM ctx bank

_CACHE = {}
REPEAT = 1                # python-unrolled repeat of the head loop
REPEAT_HW = 1             # hardware For_i repeat wrapping the head loop
_ABLATE = set()           # dev-only: subset of {"masks", "exp", "pv", "norm"}


def _build_program():
    import contextlib

    import concourse.bacc as bacc
    import concourse.bass as bass
    import concourse.mybir as mybir
    import concourse.tile as tile

    f32 = mybir.dt.float32
    bf16 = mybir.dt.bfloat16

    nc = bacc.Bacc("TRN2", target_bir_lowering=False, debug=False)

    qt_d = nc.dram_tensor("qt", [HPC, D, L], bf16, kind="ExternalInput").ap()
    kt_d = nc.dram_tensor("kt", [HPC, D, L], bf16, kind="ExternalInput").ap()
    v2_d = nc.dram_tensor("v2", [HPC, BLK, NB, V2W], bf16, kind="ExternalInput").ap()
    em_d = nc.dram_tensor("masks", [BLK, 2, BLK], bf16, kind="ExternalInput").ap()
    out_d = nc.dram_tensor("out", [HPC, BLK, NB, D], bf16, kind="ExternalOutput").ap()

    # per-kb geometry
    geo = []
    for kb in range(NB):
        K0 = BLK * kb
        qbase = K0 - W
        qlo = max(0, qbase)
        qhi = min(L, K0 + BLK + W)
        geo.append((K0, qbase, qlo, qhi))

    with tile.TileContext(nc) as tc:
        with (
            tc.tile_pool(name="const", bufs=1) as constp,
            tc.tile_pool(name="qk", bufs=2) as qkp,
            tc.tile_pool(name="pt", bufs=5) as ptp,
            tc.tile_pool(name="outb", bufs=2) as outp,
            tc.tile_pool(name="rec", bufs=2) as recp,
            tc.tile_pool(name="st", bufs=2, space="PSUM") as stp,
            tc.tile_pool(name="ctx", bufs=2, space="PSUM") as ctxp,
        ):
            em = constp.tile([BLK, 2, BLK], bf16)
            nc.sync.dma_start(out=em, in_=em_d)

            loop_cm = (
                tc.For_i(0, REPEAT_HW) if REPEAT_HW > 1 else contextlib.nullcontext()
            )
            with loop_cm:
                for h in [hh for _ in range(REPEAT) for hh in range(HPC)]:
                    qt_t = qkp.tile([D, L], bf16)
                    nc.sync.dma_start(out=qt_t, in_=qt_d[h])
                    kt_t = qkp.tile([D, L], bf16)
                    nc.sync.dma_start(out=kt_t, in_=kt_d[h])
                    v2_t = qkp.tile([BLK, NB, V2W], bf16)
                    nc.sync.dma_start(out=v2_t, in_=v2_d[h])

                    outbuf = outp.tile([BLK, NB, D], bf16)
                    pts = {}
                    ctxg = {}

                    def pv(t):
                        g, j = divmod(t, GRP)
                        if j == 0:
                            # [BLK, GRP, 128] f32 = exactly one 2KiB PSUM bank
                            ctxg[g] = ctxp.tile(
                                [BLK, GRP, BLK], f32, name="ctx_g", tag="ctx_g"
                            )
                        cg = ctxg[g]
                        klo, khi = max(0, t - 2), min(NB - 1, t + 2)
                        if "pv" in _ABLATE:
                            klo = khi
                        for kb2 in range(klo, khi + 1):
                            off = (t - kb2) * BLK + W
                            pp, s = pts[kb2]
                            nc.tensor.matmul(
                                cg[:, j, 0:V2W],
                                pp[:, s, off : off + BLK],
                                v2_t[:, kb2, :],
                                start=(kb2 == klo),
                                stop=(kb2 == khi),
                            )
                        if ("norm" not in _ABLATE) and (j == GRP - 1 or t == NB - 1):
                            n = j + 1
                            rec = recp.tile([BLK, GRP], f32)
                            nc.vector.reciprocal_approx_fast(
                                out=rec[:, 0:n], in_=cg[:, 0:n, D]
                            )
                            rec_b = bass.AP(
                                tensor=rec.tensor,
                                offset=rec.offset,
                                ap=[rec.ap[0], [1, n], [0, D]],
                            )
                            nc.vector.tensor_mul(
                                outbuf[:, t - n + 1 : t + 1, :],
                                cg[:, 0:n, 0:D],
                                rec_b,
                            )
                            del ctxg[g]

                    for kb in range(NB):
                        K0, qbase, qlo, qhi = geo[kb]
                        c_lo, c_hi = qlo - qbase, qhi - qbase
                        s = kb % 2

                        if s == 0:
                            # sub-tile stride 768 f32 = 1.5 banks so the pair
                            # tile is exactly 3 banks; matmul outputs must stay
                            # within one 2KiB PSUM bank, and bank boundaries
                            # fall at sub-tile col 512 (s=0) / 256 (s=1).
                            st = stp.tile([BLK, 2, 768], f32, name="st", tag="st")
                            pt = ptp.tile([BLK, 2, SPAN], bf16, name="pt", tag="pt")
                        split = 512 - s * 256
                        qkw = 8 if "qkmin" in _ABLATE else SPAN
                        if "qk" in _ABLATE:
                            pass
                        elif c_lo < split:
                            a, b = c_lo, min(split, c_hi, c_lo + qkw)
                            nc.tensor.matmul(
                                st[:, s, a:b],
                                kt_t[:, K0 : K0 + BLK],
                                qt_t[:, qbase + a : qbase + b],
                                start=True,
                                stop=True,
                            )
                        if "qk" not in _ABLATE and c_hi > split:
                            nc.tensor.matmul(
                                st[:, s, split : min(c_hi, split + qkw)],
                                kt_t[:, K0 : K0 + BLK],
                                qt_t[:, qbase + split : qbase + min(c_hi, split + qkw)],
                                start=True,
                                stop=True,
                            )

                        if s == 1:
                            def crange(kk):
                                K0_, qbase_, qlo_, qhi_ = geo[kk]
                                return qlo_ - qbase_, qhi_ - qbase_

                            cl0, ch0 = crange(kb - 1)
                            cl1, ch1 = crange(kb)
                            if "exp" in _ABLATE:
                                pass
                            elif (cl0, ch0) == (0, SPAN) and (cl1, ch1) == (0, SPAN):
                                # both sub-tiles full-span: one exp for the pair
                                nc.scalar.activation(
                                    pt[:, :, :],
                                    st[:, :, 0:SPAN],
                                    mybir.ActivationFunctionType.Exp,
                                    scale=float(1.0 / np.sqrt(D)),
                                )
                            else:
                                for ss, (cl, ch) in ((0, (cl0, ch0)), (1, (cl1, ch1))):
                                    nc.scalar.activation(
                                        pt[:, ss, cl:ch],
                                        st[:, ss, cl:ch],
                                        mybir.ActivationFunctionType.Exp,
                                        scale=float(1.0 / np.sqrt(D)),
                                    )

                            for kk, (cl, ch) in (
                                (kb - 1, (cl0, ch0)),
                                (kb, (cl1, ch1)),
                            ):
                                sub = pt[:, kk % 2, :]
                                if "masks" in _ABLATE:
                                    pass
                                elif (cl, ch) == (0, SPAN):
                                    pte = bass.AP(
                                        tensor=sub.tensor,
                                        offset=sub.offset,
                                        ap=[sub.ap[0], [4 * BLK, 2], [1, BLK]],
                                    )
                                    nc.vector.tensor_mul(pte, pte, em)
                                elif cl == 0:
                                    nc.vector.tensor_mul(
                                        sub[:, 0:BLK], sub[:, 0:BLK], em[:, 0, :]
                                    )
                                elif ch == SPAN:
                                    nc.vector.tensor_mul(
                                        sub[:, 4 * BLK : SPAN],
                                        sub[:, 4 * BLK : SPAN],
                                        em[:, 1, :],
                                    )
                                pts[kk] = (pt, kk % 2)

                            for tt in (kb - 3, kb - 2):
                                if tt >= 0:
                                    pv(tt)

                    pv(NB - 2)
                    pv(NB - 1)
                    pts.clear()

                    nc.sync.dma_start(out=out_d[h], in_=outbuf)

    nc.compile()
    return nc


def _get_nc():
    if "nc" not in _CACHE:
        _CACHE["nc"] = _build_program()
    return _CACHE["nc"]


def _host_prep(q, k, v):
    bf = ml_dtypes.bfloat16
    BH = B * H
    qf = (
        np.asarray(q, dtype=np.float32)
        .transpose(0, 1, 3, 2)
        .reshape(BH, D, L)
        .astype(bf)
    )
    kf = (
        np.asarray(k, dtype=np.float32)
        .transpose(0, 1, 3, 2)
        .reshape(BH, D, L)
        .astype(bf)
    )
    v2 = np.empty((BH, BLK, NB, V2W), dtype=bf)
    v2[..., :D] = (
        np.asarray(v, dtype=np.float32)
        .reshape(BH, NB, BLK, D)
        .transpose(0, 2, 1, 3)
        .astype(bf)
    )
    v2[..., D] = 1.0

    i = np.arange(BLK)
    em = np.zeros((BLK, 2, BLK), dtype=bf)
    em[:, 0, :] = (i[None, :] >= i[:, None]).astype(bf)  # left edge: col>=row
    em[:, 1, :] = (i[None, :] <= i[:, None]).astype(bf)  # right edge: col<=row

    in_maps = []
    for c in range(NCORES):
        sl = slice(c * HPC, (c + 1) * HPC)
        in_maps.append(
            {
                "qt": np.ascontiguousarray(qf[sl]),
                "kt": np.ascontiguousarray(kf[sl]),
                "v2": np.ascontiguousarray(v2[sl]),
                "masks": em,
            }
        )
    return in_maps


def kernel(q, k, v, padding_mask):
    from concourse.bass_utils import run_bass_kernel_spmd

    pm = np.asarray(padding_mask)
    assert pm.all(), "kernel specialized for all-ones padding mask"

    nc = _get_nc()
    in_maps = _host_prep(q, k, v)
    res = run_bass_kernel_spmd(nc, in_maps, core_ids=list(range(NCORES)))
    outs = [res.results[c]["out"] for c in range(NCORES)]  # (HPC, 128, 32, 64)
    full = np.concatenate(outs, axis=0)                    # (24, 128, 32, 64)
    ctx = full.transpose(0, 2, 1, 3).reshape(B, H, L, D)
    return np.ascontiguousarray(ctx.astype(np.float32))
